# revision 7
# baseline (speedup 1.0000x reference)
"""Trainium2 Bass kernel for the LayerNorm + tensor-train contraction net.

Math (per sample b):
    xn   = LayerNorm(x[b])                          # [D, L], stats over (D,L)
    t1   = xn[:,0] @ layer0                         # [R]
    t2_s = sum_{r,d} t1_r * xn[d,1] * core1[r,d,s]  # [S]
    t3_u = sum_{s,e} t2_s * xn[e,2] * core2[s,e,u]  # [U]
    out  = t3 @ last                                # [O]

Mapping (per core, pure batch data-parallel over 8 cores):
  - b-tiles of 128 samples live on SBUF partitions.
  - LN stats via bn_stats/bn_aggr in natural [b, (d l)] layout; rstd via a
    Sqrt+reciprocal; y = (x - mu) * rstd on ACT in bf16.
  - PE transposes (bf16) produce xn_T[d, b] per l; the LN affine (w,b per
    (d,l)) is folded into the PSUM->SBUF copy as per-partition scale/bias,
    which also converts to bf16.
  - TT step k: Q[b,(s,r)] = sum_d xnk_T[d,b] * Cperm[d,(s,r)] on TensorE in
    bf16 (the stationary operand is xn_T -> each output row b is that
    sample's own matvec). Q leaves PSUM as bf16 (ACT/DVE copies), then
    t_next[b,s] = sum_r t_prev[b,r] * Q[b,(s,r)] as a DVE 2x bf16
    broadcast-multiply followed by either a masked tensor_tensor_scan
    (segmented running sum; segment ends extracted by a strided copy) or a
    plain inner-axis tensor_reduce.
  - `last` is folded into core2 on the host: C2'[s,e,o] = core2 @ last.
"""

import os
import sys

import numpy as np

try:
    import concourse.bass as bass  # noqa: F401
except Exception:  # pragma: no cover - fresh-dir fallback
    for p in ("/opt/trn_rl_repo", "/root/.axon_site/_ro/trn_rl_repo"):
        if os.path.isdir(p) and p not in sys.path:
            sys.path.insert(0, p)

import concourse.bass as bass
import concourse.tile as tile
from concourse import mybir

B, D, L, R, O = 32768, 128, 3, 64, 64
S = 64
EPS = 1e-5
N_CORES = 8
BC = B // N_CORES          # samples per core
P = 128                    # partition tile (samples per b-tile)
KW = R * S                 # 4096 columns of the permuted TT cores
N_MM = 512                 # matmul free-dim per instruction
CHUNK = 1024               # q-chunk columns (2 matmuls, 2 PSUM banks)
NCHUNK = KW // CHUNK       # 4
SG = CHUNK // R            # s-groups per chunk (16)

F32 = mybir.dt.float32
BF16 = mybir.dt.bfloat16

# packed bf16 matmul-constant columns:
#   layer0 | c1p(s,r) | c2p(o,s) | identity | scan mask row
CO_L0 = 0
CO_C1 = CO_L0 + R          # 64
CO_C2 = CO_C1 + KW         # 4160
CO_ID = CO_C2 + KW         # 8256
CO_MK = CO_ID + P          # 8384
CW = CO_MK + R             # 8448
# packed fp32 scalar columns: ln_w | ln_b | eps
C2_LNW = 0
C2_LNB = C2_LNW + L
C2_EPS = C2_LNB + L
CW2 = C2_EPS + 1

# How Q leaves PSUM, per chunk index (8 chunks/tile): each char a|d assigns
# the chunk's f32->bf16 conversion to the ACT or DVE engine; "m" makes the
# broadcast-multiply read the f32 PSUM directly (no conversion).
CONV = os.environ.get("KERNEL_CONV", "aaaaaaaa")
# Segment-sum engine: masked tensor_tensor_scan vs inner-axis tensor_reduce.
RED = os.environ.get("KERNEL_RED", "reduce")

# Benchmarking aid: wrap the whole tile loop in an on-device For_i hardware
# loop running REPS extra times.
REPS = int(os.environ.get("KERNEL_REPS", "0"))


def _legalize_sync(nc, max_waits=1, max_updates=1):
    """Split multi-wait/multi-update sync_info into standalone EventSemaphore
    instructions (walrus in this env encodes at most one per instruction)."""
    import json

    bir = json.loads(mybir.module_to_json_bytes(nc.m))
    uid = [0]
    for fn in bir["functions"]:
        for blk in fn["blocks"]:
            new_insts = []
            for inst in blk["instructions"]:
                sync = inst.get("sync_info")
                if not sync:
                    new_insts.append(inst)
                    continue
                waits = sync.get("on_wait") or []
                ups = sync.get("on_update") or []
                eng = inst.get("engine")
                for w in waits[max_waits:]:
                    uid[0] += 1
                    new_insts.append({
                        "debug": inst.get("debug", 0),
                        "engine": eng,
                        "ins": [],
                        "name": f"legw-{uid[0]}",
                        "opcode": "EventSemaphore",
                        "outs": [],
                        "sync_info": {"on_update": [], "on_wait": [w]},
                    })
                sync["on_wait"] = waits[:max_waits]
                new_insts.append(inst)
                for u in ups[max_updates:]:
                    uid[0] += 1
                    new_insts.append({
                        "debug": inst.get("debug", 0),
                        "engine": eng,
                        "ins": [],
                        "name": f"legu-{uid[0]}",
                        "opcode": "EventSemaphore",
                        "outs": [],
                        "sync_info": {"on_update": [u], "on_wait": []},
                    })
                sync["on_update"] = ups[:max_updates]
            blk["instructions"] = new_insts
    nc.m = mybir.module_from_json_bytes(json.dumps(bir).encode())
    return nc


def _build_program(n_tiles: int, legalize: bool = True):
    """Emit the single-core Bass/Tile program processing n_tiles*128 samples."""
    from contextlib import ExitStack

    bc = n_tiles * P
    nc = bass.Bass()
    xs = nc.declare_dram_parameter("xs", [bc, D * L], F32, isOutput=False)
    cst = nc.declare_dram_parameter("cst", [P, CW], BF16, isOutput=False)
    cst2 = nc.declare_dram_parameter("cst2", [P, CW2], F32, isOutput=False)
    out = nc.declare_dram_parameter("out", [bc, O], F32, isOutput=True)

    Ident = mybir.ActivationFunctionType.Identity
    Sqrt = mybir.ActivationFunctionType.Sqrt
    AX = mybir.AxisListType.X
    MUL = mybir.AluOpType.mult
    ADD = mybir.AluOpType.add

    with tile.TileContext(nc) as tc, ExitStack() as ctx:
        consts = ctx.enter_context(tc.tile_pool(name="consts", bufs=1))
        xpool = ctx.enter_context(tc.tile_pool(name="xp", bufs=3))
        ypool = ctx.enter_context(tc.tile_pool(name="yp", bufs=2))
        xnt_pool = ctx.enter_context(tc.tile_pool(name="xnt", bufs=2))
        stat_pool = ctx.enter_context(tc.tile_pool(name="stats", bufs=3))
        tvec_pool = ctx.enter_context(tc.tile_pool(name="tvec", bufs=2))
        qsb_pool = ctx.enter_context(tc.tile_pool(name="qsb", bufs=2))
        ppool = ctx.enter_context(tc.tile_pool(name="pp", bufs=2))
        spool = ctx.enter_context(tc.tile_pool(name="sp", bufs=2))
        qpsum = ctx.enter_context(tc.tile_pool(name="qp", bufs=2, space="PSUM"))
        trpsum = ctx.enter_context(tc.tile_pool(name="trp", bufs=2, space="PSUM"))
        t1psum = ctx.enter_context(tc.tile_pool(name="t1p", bufs=2, space="PSUM"))

        # --- constants ---
        cst_sb = consts.tile([P, CW], BF16)
        nc.sync.dma_start(out=cst_sb[:], in_=cst[:, :])
        l0_sb = cst_sb[:, CO_L0:CO_L0 + R]
        c1p_sb = cst_sb[:, CO_C1:CO_C1 + KW]
        c2p_sb = cst_sb[:, CO_C2:CO_C2 + KW]
        ident = cst_sb[:, CO_ID:CO_ID + P]
        mask = cst_sb[:, CO_MK:CO_MK + R]
        cst2_sb = consts.tile([P, CW2], F32)
        nc.sync.dma_start(out=cst2_sb[:], in_=cst2[:, :])
        lnw_sb = cst2_sb[:, C2_LNW:C2_LNW + L]
        lnb_sb = cst2_sb[:, C2_LNB:C2_LNB + L]
        eps_sb = cst2_sb[:, C2_EPS:C2_EPS + 1]

        def tt_step(step, xnt, cperm_sb, tprev_sb, dst_sb, dst_f32):
            """dst[b, s] = sum_r tprev[b, r] * (xnt.T @ cperm)[b, (s, r)]."""
            q_sb = qsb_pool.tile([P, KW], BF16, tag=f"q{step}")
            prod = ppool.tile([P, SG * NCHUNK, R], BF16, tag=f"prod{step}")
            psum_chunks = []
            for c in range(NCHUNK):
                q = qpsum.tile([P, CHUNK], F32, tag="q")
                for n in range(CHUNK // N_MM):
                    col = c * CHUNK + n * N_MM
                    nc.tensor.matmul(
                        q[:, n * N_MM:(n + 1) * N_MM],
                        xnt[:, :],
                        cperm_sb[:, col:col + N_MM],
                        start=True,
                        stop=True,
                    )
                mode = CONV[step * NCHUNK + c]
                csl = slice(c * CHUNK, (c + 1) * CHUNK)
                if mode == "a":
                    nc.scalar.copy(q_sb[:, csl], q[:, :])
                elif mode == "d":
                    nc.vector.tensor_copy(out=q_sb[:, csl], in_=q[:, :])
                else:  # "m": multiply straight out of PSUM (f32 in, bf16 out)
                    q3 = q[:, :].rearrange("p (s r) -> p s r", r=R)
                    t_bc = tprev_sb[:, :].unsqueeze(1).broadcast_to((P, SG, R))
                    nc.vector.tensor_tensor(
                        out=prod[:, c * SG:(c + 1) * SG, :], in0=q3, in1=t_bc,
                        op=MUL)
                    psum_chunks.append(c)
            if len(psum_chunks) < NCHUNK:
                # one wide 2x multiply for all converted chunks (they are
                # contiguous in q_sb; non-"m" chunks are assumed contiguous)
                conv_cs = [c for c in range(NCHUNK) if c not in psum_chunks]
                c0, c1 = conv_cs[0], conv_cs[-1]
                ncv = c1 - c0 + 1
                q3 = q_sb[:, c0 * CHUNK:(c1 + 1) * CHUNK].rearrange(
                    "p (s r) -> p s r", r=R)
                t_bc = tprev_sb[:, :].unsqueeze(1).broadcast_to(
                    (P, ncv * SG, R))
                nc.vector.tensor_tensor(
                    out=prod[:, c0 * SG:(c1 + 1) * SG, :], in0=q3, in1=t_bc,
                    op=MUL)

            with nc.allow_low_precision(reason="bf16 TT combine"):
                if RED == "scan":
                    sc = spool.tile([P, KW], BF16, tag=f"scan{step}")
                    m_bc = mask[:, :].unsqueeze(1).broadcast_to(
                        (P, SG * NCHUNK, R)).rearrange("p s r -> p (s r)")
                    nc.vector.tensor_tensor_scan(
                        out=sc[:],
                        data0=m_bc,
                        data1=prod[:, :, :].rearrange("p s r -> p (s r)"),
                        initial=0.0,
                        op0=MUL,
                        op1=ADD,
                    )
                    # segment ends -> dst (strided extract + dtype convert)
                    ends = sc[:, :].rearrange("p (s r) -> p s r", r=R)[:, :, R - 1]
                    if dst_f32:
                        nc.vector.tensor_copy(out=dst_sb[:, :], in_=ends)
                    else:
                        nc.vector.tensor_copy(out=dst_sb[:, :], in_=ends)
                else:
                    nc.vector.tensor_reduce(
                        out=dst_sb[:, :],
                        in_=prod[:, :, :],
                        axis=AX,
                        op=ADD,
                    )

        def tile_body(i):
            row = i * P
            x_t = xpool.tile([P, D * L], F32, tag="x")
            nc.sync.dma_start(out=x_t[:], in_=xs[row:row + P, :])

            # LayerNorm stats over the full (d, l) row
            st = stat_pool.tile([P, 6], F32, tag="bn")
            nc.vector.bn_stats(out=st[:], in_=x_t[:])
            mv = stat_pool.tile([P, 2], F32, tag="mv")
            nc.vector.bn_aggr(out=mv[:], in_=st[:])
            # rstd = 1/sqrt(var + eps)
            rstd = stat_pool.tile([P, 1], F32, tag="rstd")
            nc.scalar.activation(out=rstd[:], in_=mv[:, 1:2], func=Sqrt,
                                 bias=eps_sb[:, :], scale=1.0)
            nc.vector.reciprocal(out=rstd[:], in_=rstd[:])
            # nmr = -mean * rstd
            nmr = stat_pool.tile([P, 1], F32, tag="nmr")
            nc.vector.scalar_tensor_tensor(
                out=nmr[:], in0=mv[:, 0:1], scalar=-1.0, in1=rstd[:],
                op0=MUL, op1=MUL)
            # y = (x - mean) * rstd, bf16
            y = ypool.tile([P, D * L], BF16, tag="y")
            nc.scalar.activation(out=y[:], in_=x_t[:], func=Ident,
                                 bias=nmr[:, :], scale=rstd[:, :])

            # xn_T[d, b] per l, with the affine fold on the way out of PSUM
            y3 = y[:, :].rearrange("p (d l) -> p d l", l=L)
            xnt = []
            for l in range(L):
                tr = trpsum.tile([P, P], BF16, tag="tr")
                nc.tensor.transpose(tr[:], y3[:, :, l], ident)
                xl = xnt_pool.tile([P, P], BF16, tag=f"xnt{l}")
                nc.scalar.activation(out=xl[:], in_=tr[:], func=Ident,
                                     bias=lnb_sb[:, l:l + 1],
                                     scale=lnw_sb[:, l:l + 1])
                xnt.append(xl)

            # t1[b, r] = sum_d xn0_T[d, b] * layer0[d, r]
            t1_ps = t1psum.tile([P, R], F32, tag="t1")
            nc.tensor.matmul(t1_ps[:], xnt[0][:, :], l0_sb[:, :],
                             start=True, stop=True)
            t1_sb = tvec_pool.tile([P, R], BF16, tag="t1s")
            nc.scalar.copy(t1_sb[:], t1_ps[:])

            t2_sb = tvec_pool.tile([P, S], BF16, tag="t2s")
            o_sb = tvec_pool.tile([P, O], F32, tag="os")
            tt_step(0, xnt[1], c1p_sb, t1_sb, t2_sb, False)
            tt_step(1, xnt[2], c2p_sb, t2_sb, o_sb, True)

            nc.sync.dma_start(out=out[row:row + P, :], in_=o_sb[:])

        if REPS:
            with tc.For_i(0, REPS, 1):
                for i in range(n_tiles):
                    tile_body(i)
        else:
            for i in range(n_tiles):
                tile_body(i)

    return _legalize_sync(nc) if legalize else nc


def _prep_consts(layer0, core1, core2, last, ln_w, ln_b):
    """Host-side constant massaging into packed per-dtype arrays."""
    import ml_dtypes

    layer0 = np.asarray(layer0, np.float32)
    core1 = np.asarray(core1, np.float32)
    core2 = np.asarray(core2, np.float32)
    last = np.asarray(last, np.float32)
    # c1p[d, s*R + r] = core1[r, d, s]
    c1p = core1.transpose(1, 2, 0).reshape(D, S * R)
    # fold `last` into core2: C2'[s, e, o] = sum_u core2[s,e,u] last[u,o]
    c2e = np.einsum("seu,uo->seo", core2, last).astype(np.float32)
    # c2p[e, o*S + s] = C2'[s, e, o]
    c2p = c2e.transpose(1, 2, 0).reshape(D, O * S)
    ident = np.eye(P, dtype=np.float32)
    # scan-mask row: 0 at r==0 (segment restart), 1 elsewhere
    mk = np.ones((P, R), np.float32)
    mk[:, 0] = 0.0
    packed = np.concatenate([layer0, c1p, c2p, ident, mk], axis=1)
    assert packed.shape == (P, CW), packed.shape
    packed = packed.astype(ml_dtypes.bfloat16)
    eps = np.full((P, 1), EPS, np.float32)
    packed2 = np.concatenate(
        [np.asarray(ln_w, np.float32), np.asarray(ln_b, np.float32), eps],
        axis=1)
    assert packed2.shape == (P, CW2), packed2.shape
    return {"cst": np.ascontiguousarray(packed),
            "cst2": np.ascontiguousarray(packed2)}


_cached_nc = None
last_results = None  # BassKernelResults of the most recent run (for timing)


def kernel(x, layer0, core1, core2, last, ln_w, ln_b, trace=False,
           trace_kwargs=None):
    global _cached_nc, last_results
    from concourse.bass_utils import run_bass_kernel_spmd

    x = np.asarray(x, np.float32)
    consts = _prep_consts(layer0, core1, core2, last, ln_w, ln_b)

    if _cached_nc is None:
        _cached_nc = _build_program(BC // P)
    nc = _cached_nc

    xflat = np.ascontiguousarray(x.reshape(B, D * L))
    in_maps = []
    for ci in range(N_CORES):
        m = dict(consts)
        m["xs"] = xflat[ci * BC:(ci + 1) * BC]
        in_maps.append(m)

    kw = {}
    if trace:
        kw["trace"] = True
        kw.update(trace_kwargs or {})
    res = run_bass_kernel_spmd(nc, in_maps, list(range(N_CORES)), **kw)
    last_results = res
    full = np.concatenate(
        [np.asarray(res.results[ci]["out"]) for ci in range(N_CORES)], 0)
    return np.ascontiguousarray(full.astype(np.float32, copy=False))


# revision 10
# speedup vs baseline: 1.0682x; 1.0682x over previous
"""Trainium2 Bass kernel for the LayerNorm + tensor-train contraction net.

Math (per sample b):
    xn   = LayerNorm(x[b])                          # [D, L], stats over (D,L)
    t1   = xn[:,0] @ layer0                         # [R]
    t2_s = sum_{r,d} t1_r * xn[d,1] * core1[r,d,s]  # [S]
    t3_u = sum_{s,e} t2_s * xn[e,2] * core2[s,e,u]  # [U]
    out  = t3 @ last                                # [O]

Mapping (per core, pure batch data-parallel over 8 cores):
  - b-tiles of 128 samples live on SBUF partitions.
  - LN stats via bn_stats/bn_aggr in natural [b, (d l)] layout; rstd via a
    Sqrt+reciprocal; y = (x - mu) * rstd on ACT in bf16.
  - PE transposes (bf16) produce xn_T[d, b] per l; the LN affine (w,b per
    (d,l)) is folded into the PSUM->SBUF copy as per-partition scale/bias,
    which also converts to bf16.
  - TT step k: Q[b,(s,r)] = sum_d xnk_T[d,b] * Cperm[d,(s,r)] on TensorE in
    bf16 (the stationary operand is xn_T -> each output row b is that
    sample's own matvec). Q leaves PSUM as bf16 (ACT/DVE copies), then
    t_next[b,s] = sum_r t_prev[b,r] * Q[b,(s,r)] as a DVE 2x bf16
    broadcast-multiply followed by either a masked tensor_tensor_scan
    (segmented running sum; segment ends extracted by a strided copy) or a
    plain inner-axis tensor_reduce.
  - `last` is folded into core2 on the host: C2'[s,e,o] = core2 @ last.
"""

import os
import sys

import numpy as np

try:
    import concourse.bass as bass  # noqa: F401
except Exception:  # pragma: no cover - fresh-dir fallback
    for p in ("/opt/trn_rl_repo", "/root/.axon_site/_ro/trn_rl_repo"):
        if os.path.isdir(p) and p not in sys.path:
            sys.path.insert(0, p)

import concourse.bass as bass
import concourse.tile as tile
from concourse import mybir

B, D, L, R, O = 32768, 128, 3, 64, 64
S = 64
EPS = 1e-5
N_CORES = 8
BC = B // N_CORES          # samples per core
P = 128                    # partition tile (samples per b-tile)
KW = R * S                 # 4096 columns of the permuted TT cores
N_MM = 512                 # matmul free-dim per instruction
CHUNK = 1024               # q-chunk columns (2 matmuls, 2 PSUM banks)
NCHUNK = KW // CHUNK       # 4
SG = CHUNK // R            # s-groups per chunk (16)

F32 = mybir.dt.float32
BF16 = mybir.dt.bfloat16

# packed bf16 matmul-constant columns:
#   layer0 | c1p(s,r) | c2p(o,s) | identity | scan mask row
CO_L0 = 0
CO_C1 = CO_L0 + R          # 64
CO_C2 = CO_C1 + KW         # 4160
CO_ID = CO_C2 + KW         # 8256
CO_MK = CO_ID + P          # 8384
CW = CO_MK + R             # 8448
# packed fp32 scalar columns: ln_w | ln_b | eps
C2_LNW = 0
C2_LNB = C2_LNW + L
C2_EPS = C2_LNB + L
CW2 = C2_EPS + 1

# How Q leaves PSUM, per chunk index (8 chunks/tile): each char a|d assigns
# the chunk's f32->bf16 conversion to the ACT or DVE engine; "m" makes the
# broadcast-multiply read the f32 PSUM directly (no conversion).
CONV = os.environ.get("KERNEL_CONV", "aaaaaaaa")
# Segment-sum engine: masked tensor_tensor_scan vs inner-axis tensor_reduce;
# "l1red" first halves r with a 2x bf16 add, then reduces the 32-wide rest.
RED = os.environ.get("KERNEL_RED", "reduce")
# Pool depths (overlap tuning)
QP_BUFS = int(os.environ.get("KERNEL_QP_BUFS", "2"))
TR_BUFS = int(os.environ.get("KERNEL_TR_BUFS", "2"))
SB_BUFS = int(os.environ.get("KERNEL_SB_BUFS", "2"))

# Benchmarking aid: wrap the whole tile loop in an on-device For_i hardware
# loop running REPS extra times.
REPS = int(os.environ.get("KERNEL_REPS", "0"))


def _legalize_sync(nc, max_waits=1, max_updates=1):
    """Split multi-wait/multi-update sync_info into standalone EventSemaphore
    instructions (walrus in this env encodes at most one per instruction)."""
    import json

    bir = json.loads(mybir.module_to_json_bytes(nc.m))
    uid = [0]
    for fn in bir["functions"]:
        for blk in fn["blocks"]:
            new_insts = []
            for inst in blk["instructions"]:
                sync = inst.get("sync_info")
                if not sync:
                    new_insts.append(inst)
                    continue
                waits = sync.get("on_wait") or []
                ups = sync.get("on_update") or []
                eng = inst.get("engine")
                for w in waits[max_waits:]:
                    uid[0] += 1
                    new_insts.append({
                        "debug": inst.get("debug", 0),
                        "engine": eng,
                        "ins": [],
                        "name": f"legw-{uid[0]}",
                        "opcode": "EventSemaphore",
                        "outs": [],
                        "sync_info": {"on_update": [], "on_wait": [w]},
                    })
                sync["on_wait"] = waits[:max_waits]
                new_insts.append(inst)
                for u in ups[max_updates:]:
                    uid[0] += 1
                    new_insts.append({
                        "debug": inst.get("debug", 0),
                        "engine": eng,
                        "ins": [],
                        "name": f"legu-{uid[0]}",
                        "opcode": "EventSemaphore",
                        "outs": [],
                        "sync_info": {"on_update": [u], "on_wait": []},
                    })
                sync["on_update"] = ups[:max_updates]
            blk["instructions"] = new_insts
    nc.m = mybir.module_from_json_bytes(json.dumps(bir).encode())
    return nc


def _build_program(n_tiles: int, legalize: bool = True):
    """Emit the single-core Bass/Tile program processing n_tiles*128 samples."""
    from contextlib import ExitStack

    bc = n_tiles * P
    nc = bass.Bass()
    xs = nc.declare_dram_parameter("xs", [bc, D * L], F32, isOutput=False)
    cst = nc.declare_dram_parameter("cst", [P, CW], BF16, isOutput=False)
    cst2 = nc.declare_dram_parameter("cst2", [P, CW2], F32, isOutput=False)
    out = nc.declare_dram_parameter("out", [bc, O], F32, isOutput=True)

    Ident = mybir.ActivationFunctionType.Identity
    Sqrt = mybir.ActivationFunctionType.Sqrt
    AX = mybir.AxisListType.X
    MUL = mybir.AluOpType.mult
    ADD = mybir.AluOpType.add

    with tile.TileContext(nc) as tc, ExitStack() as ctx:
        consts = ctx.enter_context(tc.tile_pool(name="consts", bufs=1))
        xpool = ctx.enter_context(tc.tile_pool(name="xp", bufs=3))
        ypool = ctx.enter_context(tc.tile_pool(name="yp", bufs=2))
        xnt_pool = ctx.enter_context(tc.tile_pool(name="xnt", bufs=2))
        stat_pool = ctx.enter_context(tc.tile_pool(name="stats", bufs=3))
        tvec_pool = ctx.enter_context(tc.tile_pool(name="tvec", bufs=2))
        qsb_pool = ctx.enter_context(tc.tile_pool(name="qsb", bufs=SB_BUFS))
        ppool = ctx.enter_context(tc.tile_pool(name="pp", bufs=SB_BUFS))
        spool = ctx.enter_context(tc.tile_pool(name="sp", bufs=2))
        qpsum = ctx.enter_context(tc.tile_pool(name="qp", bufs=QP_BUFS, space="PSUM"))
        trpsum = ctx.enter_context(tc.tile_pool(name="trp", bufs=TR_BUFS, space="PSUM"))
        t1psum = ctx.enter_context(tc.tile_pool(name="t1p", bufs=TR_BUFS, space="PSUM"))

        # --- constants ---
        cst_sb = consts.tile([P, CW], BF16)
        nc.sync.dma_start(out=cst_sb[:], in_=cst[:, :])
        l0_sb = cst_sb[:, CO_L0:CO_L0 + R]
        c1p_sb = cst_sb[:, CO_C1:CO_C1 + KW]
        c2p_sb = cst_sb[:, CO_C2:CO_C2 + KW]
        ident = cst_sb[:, CO_ID:CO_ID + P]
        mask = cst_sb[:, CO_MK:CO_MK + R]
        cst2_sb = consts.tile([P, CW2], F32)
        nc.sync.dma_start(out=cst2_sb[:], in_=cst2[:, :])
        lnw_sb = cst2_sb[:, C2_LNW:C2_LNW + L]
        lnb_sb = cst2_sb[:, C2_LNB:C2_LNB + L]
        eps_sb = cst2_sb[:, C2_EPS:C2_EPS + 1]

        def tt_step(step, xnt, cperm_sb, tprev_sb, dst_sb, dst_f32):
            """dst[b, s] = sum_r tprev[b, r] * (xnt.T @ cperm)[b, (s, r)]."""
            q_sb = qsb_pool.tile([P, KW], BF16, tag=f"q{step}")
            prod = ppool.tile([P, SG * NCHUNK, R], BF16, tag=f"prod{step}")
            psum_chunks = []
            for c in range(NCHUNK):
                q = qpsum.tile([P, CHUNK], F32, tag="q")
                for n in range(CHUNK // N_MM):
                    col = c * CHUNK + n * N_MM
                    nc.tensor.matmul(
                        q[:, n * N_MM:(n + 1) * N_MM],
                        xnt[:, :],
                        cperm_sb[:, col:col + N_MM],
                        start=True,
                        stop=True,
                    )
                mode = CONV[step * NCHUNK + c]
                csl = slice(c * CHUNK, (c + 1) * CHUNK)
                if mode == "a":
                    nc.scalar.copy(q_sb[:, csl], q[:, :])
                elif mode == "d":
                    nc.vector.tensor_copy(out=q_sb[:, csl], in_=q[:, :])
                else:  # "m": multiply straight out of PSUM (f32 in, bf16 out)
                    q3 = q[:, :].rearrange("p (s r) -> p s r", r=R)
                    t_bc = tprev_sb[:, :].unsqueeze(1).broadcast_to((P, SG, R))
                    nc.vector.tensor_tensor(
                        out=prod[:, c * SG:(c + 1) * SG, :], in0=q3, in1=t_bc,
                        op=MUL)
                    psum_chunks.append(c)
            if len(psum_chunks) < NCHUNK:
                # one wide 2x multiply for all converted chunks (they are
                # contiguous in q_sb; non-"m" chunks are assumed contiguous)
                conv_cs = [c for c in range(NCHUNK) if c not in psum_chunks]
                c0, c1 = conv_cs[0], conv_cs[-1]
                ncv = c1 - c0 + 1
                q3 = q_sb[:, c0 * CHUNK:(c1 + 1) * CHUNK].rearrange(
                    "p (s r) -> p s r", r=R)
                t_bc = tprev_sb[:, :].unsqueeze(1).broadcast_to(
                    (P, ncv * SG, R))
                nc.vector.tensor_tensor(
                    out=prod[:, c0 * SG:(c1 + 1) * SG, :], in0=q3, in1=t_bc,
                    op=MUL)

            with nc.allow_low_precision(reason="bf16 TT combine"):
                if RED == "scan":
                    sc = spool.tile([P, KW], BF16, tag=f"scan{step}")
                    m_bc = mask[:, :].unsqueeze(1).broadcast_to(
                        (P, SG * NCHUNK, R)).rearrange("p s r -> p (s r)")
                    nc.vector.tensor_tensor_scan(
                        out=sc[:],
                        data0=m_bc,
                        data1=prod[:, :, :].rearrange("p s r -> p (s r)"),
                        initial=0.0,
                        op0=MUL,
                        op1=ADD,
                    )
                    # segment ends -> dst (strided extract + dtype convert)
                    ends = sc[:, :].rearrange("p (s r) -> p s r", r=R)[:, :, R - 1]
                    if dst_f32:
                        nc.vector.tensor_copy(out=dst_sb[:, :], in_=ends)
                    else:
                        nc.vector.tensor_copy(out=dst_sb[:, :], in_=ends)
                elif RED == "l1red":
                    # halve r with one 2x bf16 add, then reduce 32-wide
                    ph = spool.tile([P, SG * NCHUNK, R // 2], BF16,
                                    tag=f"ph{step}")
                    nc.vector.tensor_tensor(
                        out=ph[:], in0=prod[:, :, 0:R // 2],
                        in1=prod[:, :, R // 2:R], op=ADD)
                    nc.vector.tensor_reduce(
                        out=dst_sb[:, :], in_=ph[:, :, :], axis=AX, op=ADD)
                else:
                    nc.vector.tensor_reduce(
                        out=dst_sb[:, :],
                        in_=prod[:, :, :],
                        axis=AX,
                        op=ADD,
                    )

        def tile_body(i):
            row = i * P
            x_t = xpool.tile([P, D * L], F32, tag="x")
            nc.sync.dma_start(out=x_t[:], in_=xs[row:row + P, :])

            # LayerNorm stats over the full (d, l) row
            st = stat_pool.tile([P, 6], F32, tag="bn")
            nc.vector.bn_stats(out=st[:], in_=x_t[:])
            mv = stat_pool.tile([P, 2], F32, tag="mv")
            nc.vector.bn_aggr(out=mv[:], in_=st[:])
            # rstd = 1/sqrt(var + eps)
            rstd = stat_pool.tile([P, 1], F32, tag="rstd")
            nc.scalar.activation(out=rstd[:], in_=mv[:, 1:2], func=Sqrt,
                                 bias=eps_sb[:, :], scale=1.0)
            nc.vector.reciprocal(out=rstd[:], in_=rstd[:])
            # nmr = -mean * rstd
            nmr = stat_pool.tile([P, 1], F32, tag="nmr")
            nc.vector.scalar_tensor_tensor(
                out=nmr[:], in0=mv[:, 0:1], scalar=-1.0, in1=rstd[:],
                op0=MUL, op1=MUL)
            # y = (x - mean) * rstd, bf16
            y = ypool.tile([P, D * L], BF16, tag="y")
            nc.scalar.activation(out=y[:], in_=x_t[:], func=Ident,
                                 bias=nmr[:, :], scale=rstd[:, :])

            # xn_T[d, b] per l, with the affine fold on the way out of PSUM
            y3 = y[:, :].rearrange("p (d l) -> p d l", l=L)
            xnt = []
            for l in range(L):
                tr = trpsum.tile([P, P], BF16, tag="tr")
                nc.tensor.transpose(tr[:], y3[:, :, l], ident)
                xl = xnt_pool.tile([P, P], BF16, tag=f"xnt{l}")
                nc.scalar.activation(out=xl[:], in_=tr[:], func=Ident,
                                     bias=lnb_sb[:, l:l + 1],
                                     scale=lnw_sb[:, l:l + 1])
                xnt.append(xl)

            # t1[b, r] = sum_d xn0_T[d, b] * layer0[d, r]
            t1_ps = t1psum.tile([P, R], F32, tag="t1")
            nc.tensor.matmul(t1_ps[:], xnt[0][:, :], l0_sb[:, :],
                             start=True, stop=True)
            t1_sb = tvec_pool.tile([P, R], BF16, tag="t1s")
            nc.scalar.copy(t1_sb[:], t1_ps[:])

            t2_sb = tvec_pool.tile([P, S], BF16, tag="t2s")
            o_sb = tvec_pool.tile([P, O], F32, tag="os")
            tt_step(0, xnt[1], c1p_sb, t1_sb, t2_sb, False)
            tt_step(1, xnt[2], c2p_sb, t2_sb, o_sb, True)

            nc.sync.dma_start(out=out[row:row + P, :], in_=o_sb[:])

        if REPS:
            with tc.For_i(0, REPS, 1):
                for i in range(n_tiles):
                    tile_body(i)
        else:
            for i in range(n_tiles):
                tile_body(i)

    return _legalize_sync(nc) if legalize else nc


def _prep_consts(layer0, core1, core2, last, ln_w, ln_b):
    """Host-side constant massaging into packed per-dtype arrays."""
    import ml_dtypes

    layer0 = np.asarray(layer0, np.float32)
    core1 = np.asarray(core1, np.float32)
    core2 = np.asarray(core2, np.float32)
    last = np.asarray(last, np.float32)
    # c1p[d, s*R + r] = core1[r, d, s]
    c1p = core1.transpose(1, 2, 0).reshape(D, S * R)
    # fold `last` into core2: C2'[s, e, o] = sum_u core2[s,e,u] last[u,o]
    c2e = np.einsum("seu,uo->seo", core2, last).astype(np.float32)
    # c2p[e, o*S + s] = C2'[s, e, o]
    c2p = c2e.transpose(1, 2, 0).reshape(D, O * S)
    ident = np.eye(P, dtype=np.float32)
    # scan-mask row: 0 at r==0 (segment restart), 1 elsewhere
    mk = np.ones((P, R), np.float32)
    mk[:, 0] = 0.0
    packed = np.concatenate([layer0, c1p, c2p, ident, mk], axis=1)
    assert packed.shape == (P, CW), packed.shape
    packed = packed.astype(ml_dtypes.bfloat16)
    eps = np.full((P, 1), EPS, np.float32)
    packed2 = np.concatenate(
        [np.asarray(ln_w, np.float32), np.asarray(ln_b, np.float32), eps],
        axis=1)
    assert packed2.shape == (P, CW2), packed2.shape
    return {"cst": np.ascontiguousarray(packed),
            "cst2": np.ascontiguousarray(packed2)}


_cached_nc = None
last_results = None  # BassKernelResults of the most recent run (for timing)


def kernel(x, layer0, core1, core2, last, ln_w, ln_b, trace=False,
           trace_kwargs=None):
    global _cached_nc, last_results
    from concourse.bass_utils import run_bass_kernel_spmd

    x = np.asarray(x, np.float32)
    consts = _prep_consts(layer0, core1, core2, last, ln_w, ln_b)

    if _cached_nc is None:
        _cached_nc = _build_program(BC // P)
    nc = _cached_nc

    xflat = np.ascontiguousarray(x.reshape(B, D * L))
    in_maps = []
    for ci in range(N_CORES):
        m = dict(consts)
        m["xs"] = xflat[ci * BC:(ci + 1) * BC]
        in_maps.append(m)

    kw = {}
    if trace:
        kw["trace"] = True
        kw.update(trace_kwargs or {})
    res = run_bass_kernel_spmd(nc, in_maps, list(range(N_CORES)), **kw)
    last_results = res
    full = np.concatenate(
        [np.asarray(res.results[ci]["out"]) for ci in range(N_CORES)], 0)
    return np.ascontiguousarray(full.astype(np.float32, copy=False))


# revision 11
# speedup vs baseline: 1.2042x; 1.1274x over previous
"""Trainium2 Bass kernel for the LayerNorm + tensor-train contraction net.

Math (per sample b):
    xn   = LayerNorm(x[b])                          # [D, L], stats over (D,L)
    t1   = xn[:,0] @ layer0                         # [R]
    t2_s = sum_{r,d} t1_r * xn[d,1] * core1[r,d,s]  # [S]
    t3_u = sum_{s,e} t2_s * xn[e,2] * core2[s,e,u]  # [U]
    out  = t3 @ last                                # [O]

Mapping (per core, pure batch data-parallel over 8 cores):
  - b-tiles of 128 samples live on SBUF partitions.
  - LN stats via bn_stats/bn_aggr in natural [b, (d l)] layout; rstd via a
    Sqrt+reciprocal; y = (x - mu) * rstd on ACT in bf16.
  - PE transposes (bf16) produce xn_T[d, b] per l; the LN affine (w,b per
    (d,l)) is folded into the PSUM->SBUF copy as per-partition scale/bias,
    which also converts to bf16.
  - TT step k: Q[b,(s,r)] = sum_d xnk_T[d,b] * Cperm[d,(s,r)] on TensorE in
    bf16 (the stationary operand is xn_T -> each output row b is that
    sample's own matvec). Q leaves PSUM as bf16 (ACT/DVE copies), then
    t_next[b,s] = sum_r t_prev[b,r] * Q[b,(s,r)] as a DVE 2x bf16
    broadcast-multiply followed by either a masked tensor_tensor_scan
    (segmented running sum; segment ends extracted by a strided copy) or a
    plain inner-axis tensor_reduce.
  - `last` is folded into core2 on the host: C2'[s,e,o] = core2 @ last.
"""

import os
import sys

import numpy as np

try:
    import concourse.bass as bass  # noqa: F401
except Exception:  # pragma: no cover - fresh-dir fallback
    for p in ("/opt/trn_rl_repo", "/root/.axon_site/_ro/trn_rl_repo"):
        if os.path.isdir(p) and p not in sys.path:
            sys.path.insert(0, p)

import concourse.bass as bass
import concourse.tile as tile
from concourse import mybir

B, D, L, R, O = 32768, 128, 3, 64, 64
S = 64
EPS = 1e-5
N_CORES = 8
BC = B // N_CORES          # samples per core
P = 128                    # partition tile (samples per b-tile)
KW = R * S                 # 4096 columns of the permuted TT cores
N_MM = 512                 # matmul free-dim per instruction
CHUNK = 1024               # q-chunk columns (2 matmuls, 2 PSUM banks)
NCHUNK = KW // CHUNK       # 4
SG = CHUNK // R            # s-groups per chunk (16)

F32 = mybir.dt.float32
BF16 = mybir.dt.bfloat16

# packed bf16 matmul-constant columns:
#   layer0 | c1p(s,r) | c2p(o,s) | identity | scan mask row
CO_L0 = 0
CO_C1 = CO_L0 + R          # 64
CO_C2 = CO_C1 + KW         # 4160
CO_ID = CO_C2 + KW         # 8256
CO_MK = CO_ID + P          # 8384
CW = CO_MK + R             # 8448
# packed fp32 scalar columns: ln_w | ln_b | eps
C2_LNW = 0
C2_LNB = C2_LNW + L
C2_EPS = C2_LNB + L
CW2 = C2_EPS + 1

# How Q leaves PSUM, per chunk index (8 chunks/tile): each char a|d assigns
# the chunk's f32->bf16 conversion to the ACT or DVE engine; "m" makes the
# broadcast-multiply read the f32 PSUM directly (no conversion).
CONV = os.environ.get("KERNEL_CONV", "aaaaaaaa")
# Segment-sum engine: masked tensor_tensor_scan vs inner-axis tensor_reduce;
# "l1red" first halves r with a 2x bf16 add, then reduces the 32-wide rest.
RED = os.environ.get("KERNEL_RED", "reduce")
# Pool depths (overlap tuning)
QP_BUFS = int(os.environ.get("KERNEL_QP_BUFS", "2"))
TR_BUFS = int(os.environ.get("KERNEL_TR_BUFS", "2"))
SB_BUFS = int(os.environ.get("KERNEL_SB_BUFS", "2"))

# Benchmarking aid: wrap the whole tile loop in an on-device For_i hardware
# loop running REPS extra times.
REPS = int(os.environ.get("KERNEL_REPS", "0"))


def _legalize_sync(nc, max_waits=1, max_updates=1):
    """Split multi-wait/multi-update sync_info into standalone EventSemaphore
    instructions (walrus in this env encodes at most one per instruction)."""
    import json

    bir = json.loads(mybir.module_to_json_bytes(nc.m))
    uid = [0]
    for fn in bir["functions"]:
        for blk in fn["blocks"]:
            new_insts = []
            for inst in blk["instructions"]:
                sync = inst.get("sync_info")
                if not sync:
                    new_insts.append(inst)
                    continue
                waits = sync.get("on_wait") or []
                ups = sync.get("on_update") or []
                eng = inst.get("engine")
                for w in waits[max_waits:]:
                    uid[0] += 1
                    new_insts.append({
                        "debug": inst.get("debug", 0),
                        "engine": eng,
                        "ins": [],
                        "name": f"legw-{uid[0]}",
                        "opcode": "EventSemaphore",
                        "outs": [],
                        "sync_info": {"on_update": [], "on_wait": [w]},
                    })
                sync["on_wait"] = waits[:max_waits]
                new_insts.append(inst)
                for u in ups[max_updates:]:
                    uid[0] += 1
                    new_insts.append({
                        "debug": inst.get("debug", 0),
                        "engine": eng,
                        "ins": [],
                        "name": f"legu-{uid[0]}",
                        "opcode": "EventSemaphore",
                        "outs": [],
                        "sync_info": {"on_update": [u], "on_wait": []},
                    })
                sync["on_update"] = ups[:max_updates]
            blk["instructions"] = new_insts
    nc.m = mybir.module_from_json_bytes(json.dumps(bir).encode())
    return nc


def _build_program(n_tiles: int, legalize: bool = True):
    """Emit the single-core Bass/Tile program processing n_tiles*128 samples."""
    from contextlib import ExitStack

    bc = n_tiles * P
    nc = bass.Bass()
    xs = nc.declare_dram_parameter("xs", [bc, D * L], F32, isOutput=False)
    cst = nc.declare_dram_parameter("cst", [P, CW], BF16, isOutput=False)
    cst2 = nc.declare_dram_parameter("cst2", [P, CW2], F32, isOutput=False)
    out = nc.declare_dram_parameter("out", [bc, O], F32, isOutput=True)

    Ident = mybir.ActivationFunctionType.Identity
    Sqrt = mybir.ActivationFunctionType.Sqrt
    AX = mybir.AxisListType.X
    MUL = mybir.AluOpType.mult
    ADD = mybir.AluOpType.add

    with tile.TileContext(nc) as tc, ExitStack() as ctx:
        consts = ctx.enter_context(tc.tile_pool(name="consts", bufs=1))
        xpool = ctx.enter_context(tc.tile_pool(name="xp", bufs=3))
        ypool = ctx.enter_context(tc.tile_pool(name="yp", bufs=2))
        xnt_pool = ctx.enter_context(tc.tile_pool(name="xnt", bufs=2))
        stat_pool = ctx.enter_context(tc.tile_pool(name="stats", bufs=3))
        tvec_pool = ctx.enter_context(tc.tile_pool(name="tvec", bufs=2))
        qsb_pool = ctx.enter_context(tc.tile_pool(name="qsb", bufs=SB_BUFS))
        ppool = ctx.enter_context(tc.tile_pool(name="pp", bufs=SB_BUFS))
        spool = ctx.enter_context(tc.tile_pool(name="sp", bufs=2))
        qpsum = ctx.enter_context(tc.tile_pool(name="qp", bufs=QP_BUFS, space="PSUM"))
        trpsum = ctx.enter_context(tc.tile_pool(name="trp", bufs=TR_BUFS, space="PSUM"))
        t1psum = ctx.enter_context(tc.tile_pool(name="t1p", bufs=TR_BUFS, space="PSUM"))

        # --- constants ---
        cst_sb = consts.tile([P, CW], BF16)
        nc.sync.dma_start(out=cst_sb[:], in_=cst[:, :])
        l0_sb = cst_sb[:, CO_L0:CO_L0 + R]
        c1p_sb = cst_sb[:, CO_C1:CO_C1 + KW]
        c2p_sb = cst_sb[:, CO_C2:CO_C2 + KW]
        ident = cst_sb[:, CO_ID:CO_ID + P]
        mask = cst_sb[:, CO_MK:CO_MK + R]
        cst2_sb = consts.tile([P, CW2], F32)
        nc.sync.dma_start(out=cst2_sb[:], in_=cst2[:, :])
        lnw_sb = cst2_sb[:, C2_LNW:C2_LNW + L]
        lnb_sb = cst2_sb[:, C2_LNB:C2_LNB + L]
        eps_sb = cst2_sb[:, C2_EPS:C2_EPS + 1]

        def tt_step(step, xnt, cperm_sb, tprev_sb, dst_sb, dst_f32):
            """dst[b, s] = sum_r tprev[b, r] * (xnt.T @ cperm)[b, (s, r)]."""
            q_sb = qsb_pool.tile([P, KW], BF16, tag=f"q{step}")
            prod = ppool.tile([P, SG * NCHUNK, R], BF16, tag=f"prod{step}")
            psum_chunks = []
            for c in range(NCHUNK):
                q = qpsum.tile([P, CHUNK], F32, tag="q")
                for n in range(CHUNK // N_MM):
                    col = c * CHUNK + n * N_MM
                    nc.tensor.matmul(
                        q[:, n * N_MM:(n + 1) * N_MM],
                        xnt[:, :],
                        cperm_sb[:, col:col + N_MM],
                        start=True,
                        stop=True,
                    )
                mode = CONV[step * NCHUNK + c]
                csl = slice(c * CHUNK, (c + 1) * CHUNK)
                if mode == "a":
                    nc.scalar.copy(q_sb[:, csl], q[:, :])
                elif mode == "d":
                    nc.vector.tensor_copy(out=q_sb[:, csl], in_=q[:, :])
                else:  # "m": multiply straight out of PSUM (f32 in, bf16 out)
                    q3 = q[:, :].rearrange("p (s r) -> p s r", r=R)
                    t_bc = tprev_sb[:, :].unsqueeze(1).broadcast_to((P, SG, R))
                    nc.vector.tensor_tensor(
                        out=prod[:, c * SG:(c + 1) * SG, :], in0=q3, in1=t_bc,
                        op=MUL)
                    psum_chunks.append(c)
            if len(psum_chunks) < NCHUNK:
                # one wide 2x multiply for all converted chunks (they are
                # contiguous in q_sb; non-"m" chunks are assumed contiguous)
                conv_cs = [c for c in range(NCHUNK) if c not in psum_chunks]
                c0, c1 = conv_cs[0], conv_cs[-1]
                ncv = c1 - c0 + 1
                q3 = q_sb[:, c0 * CHUNK:(c1 + 1) * CHUNK].rearrange(
                    "p (s r) -> p s r", r=R)
                t_bc = tprev_sb[:, :].unsqueeze(1).broadcast_to(
                    (P, ncv * SG, R))
                nc.vector.tensor_tensor(
                    out=prod[:, c0 * SG:(c1 + 1) * SG, :], in0=q3, in1=t_bc,
                    op=MUL)

            with nc.allow_low_precision(reason="bf16 TT combine"):
                if RED == "scan":
                    sc = spool.tile([P, KW], BF16, tag=f"scan{step}")
                    m_bc = mask[:, :].unsqueeze(1).broadcast_to(
                        (P, SG * NCHUNK, R)).rearrange("p s r -> p (s r)")
                    nc.vector.tensor_tensor_scan(
                        out=sc[:],
                        data0=m_bc,
                        data1=prod[:, :, :].rearrange("p s r -> p (s r)"),
                        initial=0.0,
                        op0=MUL,
                        op1=ADD,
                    )
                    # segment ends -> dst (strided extract + dtype convert)
                    ends = sc[:, :].rearrange("p (s r) -> p s r", r=R)[:, :, R - 1]
                    if dst_f32:
                        nc.vector.tensor_copy(out=dst_sb[:, :], in_=ends)
                    else:
                        nc.vector.tensor_copy(out=dst_sb[:, :], in_=ends)
                elif RED == "l1red":
                    # halve r with one 2x bf16 add, then reduce 32-wide
                    ph = spool.tile([P, SG * NCHUNK, R // 2], BF16,
                                    tag=f"ph{step}")
                    nc.vector.tensor_tensor(
                        out=ph[:], in0=prod[:, :, 0:R // 2],
                        in1=prod[:, :, R // 2:R], op=ADD)
                    nc.vector.tensor_reduce(
                        out=dst_sb[:, :], in_=ph[:, :, :], axis=AX, op=ADD)
                elif RED == "l2red":
                    # two 2x bf16 halving adds, then reduce 16-wide
                    ph = spool.tile([P, SG * NCHUNK, R // 2], BF16,
                                    tag=f"ph{step}")
                    nc.vector.tensor_tensor(
                        out=ph[:], in0=prod[:, :, 0:R // 2],
                        in1=prod[:, :, R // 2:R], op=ADD)
                    pq = spool.tile([P, SG * NCHUNK, R // 4], BF16,
                                    tag=f"pq{step}")
                    nc.vector.tensor_tensor(
                        out=pq[:], in0=ph[:, :, 0:R // 4],
                        in1=ph[:, :, R // 4:R // 2], op=ADD)
                    nc.vector.tensor_reduce(
                        out=dst_sb[:, :], in_=pq[:, :, :], axis=AX, op=ADD)
                else:
                    nc.vector.tensor_reduce(
                        out=dst_sb[:, :],
                        in_=prod[:, :, :],
                        axis=AX,
                        op=ADD,
                    )

        def tile_body(i):
            row = i * P
            x_t = xpool.tile([P, D * L], F32, tag="x")
            nc.sync.dma_start(out=x_t[:], in_=xs[row:row + P, :])

            # LayerNorm stats over the full (d, l) row
            st = stat_pool.tile([P, 6], F32, tag="bn")
            nc.vector.bn_stats(out=st[:], in_=x_t[:])
            mv = stat_pool.tile([P, 2], F32, tag="mv")
            nc.vector.bn_aggr(out=mv[:], in_=st[:])
            # rstd = 1/sqrt(var + eps)
            rstd = stat_pool.tile([P, 1], F32, tag="rstd")
            nc.scalar.activation(out=rstd[:], in_=mv[:, 1:2], func=Sqrt,
                                 bias=eps_sb[:, :], scale=1.0)
            nc.vector.reciprocal(out=rstd[:], in_=rstd[:])
            # nmr = -mean * rstd
            nmr = stat_pool.tile([P, 1], F32, tag="nmr")
            nc.vector.scalar_tensor_tensor(
                out=nmr[:], in0=mv[:, 0:1], scalar=-1.0, in1=rstd[:],
                op0=MUL, op1=MUL)
            # y = (x - mean) * rstd, bf16
            y = ypool.tile([P, D * L], BF16, tag="y")
            nc.scalar.activation(out=y[:], in_=x_t[:], func=Ident,
                                 bias=nmr[:, :], scale=rstd[:, :])

            # xn_T[d, b] per l, with the affine fold on the way out of PSUM
            y3 = y[:, :].rearrange("p (d l) -> p d l", l=L)
            xnt = []
            for l in range(L):
                tr = trpsum.tile([P, P], BF16, tag="tr")
                nc.tensor.transpose(tr[:], y3[:, :, l], ident)
                xl = xnt_pool.tile([P, P], BF16, tag=f"xnt{l}")
                nc.scalar.activation(out=xl[:], in_=tr[:], func=Ident,
                                     bias=lnb_sb[:, l:l + 1],
                                     scale=lnw_sb[:, l:l + 1])
                xnt.append(xl)

            # t1[b, r] = sum_d xn0_T[d, b] * layer0[d, r]
            t1_ps = t1psum.tile([P, R], F32, tag="t1")
            nc.tensor.matmul(t1_ps[:], xnt[0][:, :], l0_sb[:, :],
                             start=True, stop=True)
            t1_sb = tvec_pool.tile([P, R], BF16, tag="t1s")
            nc.scalar.copy(t1_sb[:], t1_ps[:])

            t2_sb = tvec_pool.tile([P, S], BF16, tag="t2s")
            o_sb = tvec_pool.tile([P, O], F32, tag="os")
            tt_step(0, xnt[1], c1p_sb, t1_sb, t2_sb, False)
            tt_step(1, xnt[2], c2p_sb, t2_sb, o_sb, True)

            nc.sync.dma_start(out=out[row:row + P, :], in_=o_sb[:])

        if REPS:
            with tc.For_i(0, REPS, 1):
                for i in range(n_tiles):
                    tile_body(i)
        else:
            for i in range(n_tiles):
                tile_body(i)

    return _legalize_sync(nc) if legalize else nc


def _prep_consts(layer0, core1, core2, last, ln_w, ln_b):
    """Host-side constant massaging into packed per-dtype arrays."""
    import ml_dtypes

    layer0 = np.asarray(layer0, np.float32)
    core1 = np.asarray(core1, np.float32)
    core2 = np.asarray(core2, np.float32)
    last = np.asarray(last, np.float32)
    # c1p[d, s*R + r] = core1[r, d, s]
    c1p = core1.transpose(1, 2, 0).reshape(D, S * R)
    # fold `last` into core2: C2'[s, e, o] = sum_u core2[s,e,u] last[u,o]
    c2e = np.einsum("seu,uo->seo", core2, last).astype(np.float32)
    # c2p[e, o*S + s] = C2'[s, e, o]
    c2p = c2e.transpose(1, 2, 0).reshape(D, O * S)
    ident = np.eye(P, dtype=np.float32)
    # scan-mask row: 0 at r==0 (segment restart), 1 elsewhere
    mk = np.ones((P, R), np.float32)
    mk[:, 0] = 0.0
    packed = np.concatenate([layer0, c1p, c2p, ident, mk], axis=1)
    assert packed.shape == (P, CW), packed.shape
    packed = packed.astype(ml_dtypes.bfloat16)
    eps = np.full((P, 1), EPS, np.float32)
    packed2 = np.concatenate(
        [np.asarray(ln_w, np.float32), np.asarray(ln_b, np.float32), eps],
        axis=1)
    assert packed2.shape == (P, CW2), packed2.shape
    return {"cst": np.ascontiguousarray(packed),
            "cst2": np.ascontiguousarray(packed2)}


_cached_nc = None
last_results = None  # BassKernelResults of the most recent run (for timing)


def kernel(x, layer0, core1, core2, last, ln_w, ln_b, trace=False,
           trace_kwargs=None):
    global _cached_nc, last_results
    from concourse.bass_utils import run_bass_kernel_spmd

    x = np.asarray(x, np.float32)
    consts = _prep_consts(layer0, core1, core2, last, ln_w, ln_b)

    if _cached_nc is None:
        _cached_nc = _build_program(BC // P)
    nc = _cached_nc

    xflat = np.ascontiguousarray(x.reshape(B, D * L))
    in_maps = []
    for ci in range(N_CORES):
        m = dict(consts)
        m["xs"] = xflat[ci * BC:(ci + 1) * BC]
        in_maps.append(m)

    kw = {}
    if trace:
        kw["trace"] = True
        kw.update(trace_kwargs or {})
    res = run_bass_kernel_spmd(nc, in_maps, list(range(N_CORES)), **kw)
    last_results = res
    full = np.concatenate(
        [np.asarray(res.results[ci]["out"]) for ci in range(N_CORES)], 0)
    return np.ascontiguousarray(full.astype(np.float32, copy=False))
